# revision 5
# baseline (speedup 1.0000x reference)
"""Biased multi-head self-attention (B=4, N=1024, H=1024, 16 heads) on 8
Trainium2 NeuronCores.

Sharding: data-parallel over batch (4) x tensor-parallel over head-groups
(2 groups of 8 heads) = 8 cores. Core c handles batch c//2, head-group c%2.
Host sums the two head-groups' partial output projections per batch and
adds bp. 168.7us vs the 218.5us v1 baseline.

Design (all engine-load numbers per core):
  - attn_bias is folded in multiplicatively: host precomputes
    exp(attn_bias^T + c[m]) in fp16 (c[m] = k0[m]@bq, the only survivor of
    the q/k projection biases under the softmax; q0@bk and bq@bk are
    constant along the reduced axis and cancel). On device
    et = exp(qk) * eb via fp16 SBUF tensor_tensor on DVE (2x mode) —
    this removes the v1 identity-matmul bias injection (25% of PE cycles).
  - EXCEPT for the first two m-chunks of each pair, which keep the v1
    additive path (identity-matmul + exp(s+b), raw bias rows shipped in the
    same tensor): those iterations then have no DVE/Pool dependency, so the
    next pair never stalls behind the previous pair's norm chain
    draining through the in-order DVE/Pool queues.
  - PV matmuls run TWO iterations behind their score->exp->mult chain and
    the NEXT pair's q/k projection matmuls interleave as PE filler: the PE
    never idles mid-pair (idle PE drops to a 1.2GHz p-state on TRN2 and
    poisons several us of subsequent matmuls).
  - PSUM: 3 score slots + 1 projection slot shared in one tag rotation is
    avoided; tags "s"(3) + "f"(5) = 8 banks; the 5-deep f rotation staggers
    PV-accumulator reuse so each unit waits only one (already finished)
    norm, not the whole previous pair.
  - Softmax denominators ride an interleaved ones-column in the V
    projection (PV row 64); normalization = denominator row staged to SBUF
    (DVE), reciprocal_approx_fast (DVE, 5x faster than reciprocal),
    partition_broadcast (Pool), multiply (DVE) — emitted phase-batched so
    units pipeline across engines instead of serializing.
  - Tail: after the last pair, first-half feats are normalized, the 8
    output-projection groups that need only them run while the second-half
    norm drains, then the rest. Output is fp16 (host sums partials fp32).
  - x/wv staged in per-chunk DMAs so the V projection starts as soon as
    chunk 0 lands; eb (bias) chunks prefetched one pair ahead on the sync
    queue; wp deferred; all big stage-in on sync, in consumption order.

Hardware facts this leans on (measured/verified this session):
  - matmul time = out-free-size cycles @2.4GHz regardless of K and M; a
    [128,512] fp32 K=1 broadcast matmul costs 4 cyc/row (fp32 penalty).
  - GPSIMD/Pool cannot touch PSUM at all; custom-DVE (ISA) ops need
    partition-0-aligned operands and misread PSUM sources; regular engine
    ops need 32-aligned PSUM partition bases.
  - fp32r matmul inputs must come from an op that rounds to fp32r.
  - DVE tensor_tensor: 2x only with all-16-bit packed SBUF operands; any
    PSUM operand forces 1x fp32 rates.
"""

import sys

for _p in ("/opt/trn_rl_repo", "/opt/pypackages"):
    if _p not in sys.path:
        sys.path.append(_p)

import numpy as np

import concourse.bass as bass
import concourse.bacc as bacc
import concourse.mybir as mybir
import concourse.tile as tile
from concourse.bass_utils import run_bass_kernel_spmd

P = 128
N = 1024          # sequence length
H = 1024          # model dim
B = 4
NH = 16
HS = 64
G = 2             # head groups (tensor parallel)
HL = NH // G      # heads per core = 8
DLOC = H // G     # feature cols per core = 512
DAUG = HL * (HS + 1) + HL   # 528: v + ones col per head + c col per head
CBASE = HL * (HS + 1)       # 520: where the c columns start
HC = H // P       # 8 contraction chunks over model dim
DC = DLOC // P    # 4 chunks over local feature dim (= head pairs)
NB = N // 512     # 2 n blocks
NT = N // P       # 8 n tiles
MC = N // P       # 8 m chunks
HF = DAUG // 2    # 264 v-projection half width
SCALE = 1.0 / np.sqrt(HS)

F32 = mybir.dt.float32
F32R = mybir.dt.float32r
F16 = mybir.dt.float16
Act = mybir.ActivationFunctionType
Alu = mybir.AluOpType

_PROG = None


def _emit(nc, tc, io):
    xT, ebT, wq, wk, wv, wp, bv, ones, onesb2, ident_d, out = io

    import contextlib

    with contextlib.ExitStack() as ctx:
        consts = ctx.enter_context(tc.tile_pool(name="consts", bufs=1))
        stage = ctx.enter_context(tc.tile_pool(name="stage", bufs=1))
        qkv = ctx.enter_context(tc.tile_pool(name="qkv", bufs=1))
        et0_pool = ctx.enter_context(tc.tile_pool(name="et0", bufs=6))
        et_pool = ctx.enter_context(tc.tile_pool(name="et", bufs=16))
        eb_pool = ctx.enter_context(tc.tile_pool(name="eb", bufs=16))
        inv_pool = ctx.enter_context(tc.tile_pool(name="inv", bufs=4))
        bs_pool = ctx.enter_context(tc.tile_pool(name="bs", bufs=4))
        opool = ctx.enter_context(tc.tile_pool(name="opool", bufs=3))
        # one PSUM pool, 8 banks total: 3 score slots + 1 projection slot +
        # 4 PV accumulators
        psp = ctx.enter_context(tc.tile_pool(name="psp", bufs=1, space="PSUM"))

        ones_t = consts.tile([1, P], F16)
        nc.gpsimd.dma_start(out=ones_t, in_=ones)
        onesb2_t = consts.tile([2, P], F32)
        nc.gpsimd.dma_start(out=onesb2_t, in_=onesb2)
        bv_sb = consts.tile([1, DAUG], F16)
        nc.gpsimd.dma_start(out=bv_sb, in_=bv)
        ident_t = consts.tile([P, P], F16)
        nc.gpsimd.dma_start(out=ident_t, in_=ident_d)

        # input staging: x and wv interleaved in 2-chunk pieces so the v
        # projection starts on chunk 0 while later chunks stream in.
        xr = xT.rearrange("(c p) n -> p c n", p=P)
        wvr = wv.rearrange("(c p) d -> p c d", p=P)
        xc = []
        wvc = []
        for i in range(4):
            xt = stage.tile([P, 2, N], F16, name=f"xc{i}")
            wt = stage.tile([P, 2, DAUG], F16, name=f"wvc{i}")
            nc.sync.dma_start(out=xt, in_=xr[:, 2 * i : 2 * i + 2])
            nc.sync.dma_start(out=wt, in_=wvr[:, 2 * i : 2 * i + 2])
            xc.append(xt)
            wvc.append(wt)
        wq_m = stage.tile([P, HC, DLOC], F16, name="wqm")
        nc.sync.dma_start(out=wq_m, in_=wq.rearrange("(c p) d -> p c d", p=P))
        wk_m = stage.tile([P, HC, DLOC], F16, name="wkm")
        nc.sync.dma_start(out=wk_m, in_=wk.rearrange("(c p) d -> p c d", p=P))
        wp_sb = stage.tile([P, DC, H], F16, name="wpm")
        nc.sync.dma_start(out=wp_sb, in_=wp.rearrange("(c p) o -> p c o", p=P))
        xT_t = [xc[hc // 2][:, hc % 2] for hc in range(HC)]
        wv_t = [wvc[hc // 2][:, hc % 2] for hc in range(HC)]

        v_sb = qkv.tile([P, MC, DAUG], F16)
        c_sb = qkv.tile([P, MC, HL], F32)
        qT_sb = qkv.tile([P, DC, N], F16)
        kT_sb = qkv.tile([P, DC, N], F16)
        featsT_sb = qkv.tile([P, DC, N], F16)

        # eb (exp of attn bias, transposed) chunk loads: [P, 2 mc, N] per
        # (head, mc-pair); one full pair (8 chunks) is prefetched ahead.
        eb_t = {}

        def load_eb(h, mcp):
            bt = eb_pool.tile([P, 2, N], F16, tag="eb", name=f"eb{h}_{mcp}")
            nc.sync.dma_start(
                out=bt,
                in_=ebT[h].rearrange("(c p) n -> p c n", p=P)[
                    :, 2 * mcp : 2 * mcp + 2
                ],
            )
            eb_t[(h, mcp)] = bt

        for h in (0, 1):  # pair 0 prefetch (first half; rest mid-loop)
            for mcp in range(2):
                load_eb(h, mcp)
        # pool holds 16 chunk tiles = exactly two pairs in flight

        # ---- v projection (+ bv and per-head ones column and c columns) ----
        # two waves of 8 psum tiles, chunk-paced over the contraction dim
        for wave in range(2):
            tiles = []
            wtags = ["s", "s", "s", "f", "f", "f", "f", "f"]
            wbufs = {"s": 3, "f": 5}
            for i, (nt, half) in enumerate(
                (nt, half) for nt in range(4 * wave, 4 * wave + 4) for half in range(2)
            ):
                tg = wtags[i]
                ps = psp.tile(
                    [P, HF], F32, tag=tg, bufs=wbufs[tg], name=f"vps{nt}_{half}"
                )
                tiles.append((nt, half, ps))
            for hc in range(HC):
                for nt, half, ps in tiles:
                    nc.tensor.matmul(
                        ps,
                        (xT_t[hc][:, nt * P : (nt + 1) * P]),
                        (wv_t[hc][:, half * HF : (half + 1) * HF]),
                        start=(hc == 0),
                        stop=False,
                    )
            for nt, half, ps in tiles:
                nc.tensor.matmul(
                    ps,
                    (ones_t[:1, :P]),
                    (bv_sb[:1, half * HF : (half + 1) * HF]),
                    start=False,
                    stop=True,
                )
                nc.scalar.copy(v_sb[:, nt, half * HF : (half + 1) * HF], ps)
        # c columns (the k0@bq exp-bias per head) to fp32 for the ACT bias arg
        nc.scalar.copy(c_sb, v_sb[:, :, CBASE : CBASE + HL])

        # ---- attention: software-pipelined head pairs ----
        # Per pair: one mc-outer loop with 4 units (2 heads x 2 nb) per
        # iteration. The PV matmuls run one iteration BEHIND their exp/mult
        # chain (always-ready PE work), and the NEXT pair's q/k projection
        # matmuls are interleaved as PE filler so the PE never idles while
        # the scalar engine churns exps (idle PE drops to a lower p-state).
        def proj_ops(hp):
            """Flat list of closures: 4 projection units (k/q x nb), each 8
            accumulating matmuls + one DVE evacuation, into the "pj" slot."""
            ops = []
            for w_m, dst, nb in (
                (wk_m, kT_sb, 0),
                (wq_m, qT_sb, 0),
                (wk_m, kT_sb, 1),
                (wq_m, qT_sb, 1),
            ):
                box = {}

                def mk_mm(hc, w_m=w_m, nb=nb, box=box):
                    def op():
                        if hc == 0:
                            box["ps"] = psp.tile(
                                [P, 512], F32, tag="s", bufs=3, name=f"qk{hp}{nb}"
                            )
                        nc.tensor.matmul(
                            box["ps"],
                            (w_m[:, hc, hp * P : (hp + 1) * P]),
                            (xT_t[hc][:, nb * 512 : (nb + 1) * 512]),
                            start=(hc == 0),
                            stop=(hc == HC - 1),
                        )
                    return op

                def mk_evac(dst=dst, nb=nb, box=box):
                    def op():
                        nc.vector.tensor_copy(
                            dst[:, hp, nb * 512 : (nb + 1) * 512], box["ps"]
                        )
                    return op

                ops.extend(mk_mm(hc) for hc in range(HC))
                ops.append(mk_evac())
            return ops

        o_ts = {}

        def emit_group(nt, cb, tg, tb, evac_dve=False):
            ps = psp.tile([P, 512], F32, tag=tg, bufs=tb, name=f"og{nt}_{cb}")
            for dc in range(DC):
                nc.tensor.matmul(
                    ps,
                    (featsT_sb[:, dc, nt * P : (nt + 1) * P]),
                    (wp_sb[:, dc, cb * 512 : (cb + 1) * 512]),
                    start=(dc == 0),
                    stop=(dc == DC - 1),
                )
            if nt not in o_ts:
                o_ts[nt] = opool.tile([P, N], F16, tag="o", name=f"ot{nt}")
            if evac_dve:
                nc.vector.tensor_copy(o_ts[nt][:, cb * 512 : (cb + 1) * 512], ps)
            else:
                nc.scalar.copy(o_ts[nt][:, cb * 512 : (cb + 1) * 512], ps)
            if cb == 1:
                eng = [nc.sync, nc.scalar, nc.gpsimd][nt % 3]
                eng.dma_start(out=out[nt * P : (nt + 1) * P, :], in_=o_ts[nt])

        def emit_norm(hp, unit_list, f_ps, inv32, srow_dve=False):
            """Batched: all srows (ACT, or DVE when ACT is backlogged), all
            reciprocals (DVE), all partition broadcasts (Pool), then all
            normalize multiplies (DVE) — no per-unit serialization."""
            b_sbs = {}
            srows = {}
            for h, nb in unit_list:
                srow = inv_pool.tile([1, 512], F32, tag="srow", name=f"sr{h}{nb}")
                nc.vector.tensor_copy(srow, f_ps[(h, nb)][HS : HS + 1, :])
                srows[(h, nb)] = srow
            for h, nb in unit_list:
                nc.vector.reciprocal_approx_fast(
                    out=inv32[h][:1, nb * 512 : (nb + 1) * 512],
                    in_=srows[(h, nb)],
                )
            for h, nb in unit_list:
                b_sb = bs_pool.tile([HS, 512], F32, tag="bs", name=f"bs{h}{nb}")
                nc.gpsimd.partition_broadcast(
                    b_sb, inv32[h][:1, nb * 512 : (nb + 1) * 512]
                )
                b_sbs[(h, nb)] = b_sb
            for h, nb in unit_list:
                po = (h % 2) * HS
                nc.vector.tensor_mul(
                    out=featsT_sb[po : po + HS, hp, nb * 512 : (nb + 1) * 512],
                    in0=f_ps[(h, nb)][:HS, :],
                    in1=b_sbs[(h, nb)],
                )

        # pair 0's projection runs up front
        for op in proj_ops(0):
            op()

        for hp in range(4):
            heads = (2 * hp, 2 * hp + 1)
            units = [(h, nb) for nb in range(NB) for h in heads]
            filler = proj_ops(hp + 1) if hp < 3 else []
            fpos = 0
            inv32 = {
                h: inv_pool.tile([1, N], F32, tag="inv", name=f"inv{hp}_{h}")
                for h in heads
            }
            f_ps = {
                (h, nb): psp.tile(
                    [HS + 1, 512], F32, tag="f", bufs=5, name=f"fps{h}_{nb}"
                )
                for h, nb in units
            }
            prev_ets = None

            def score_chain(h, nb, mc):
                # mc 0..1: additive bias via an identity matmul into PSUM
                # (PE+ACT only — keeps the pair boundary free of DVE/Pool
                # queue dependencies). mc 2..7: multiplicative exp(bias).
                dpo = (h % 2) * HS
                sp = psp.tile([P, 512], F32, tag="s", bufs=3, name=f"sp{h}_{mc}_{nb}")
                ident_path = mc < 2 and hp > 0  # pair 0 has no predecessor
                if ident_path:
                    nc.tensor.matmul(
                        sp,
                        ident_t,
                        eb_t[(h, mc // 2)][:, mc % 2, nb * 512 : (nb + 1) * 512],
                        start=True,
                        stop=False,
                    )
                nc.tensor.matmul(
                    sp,
                    kT_sb[dpo : dpo + HS, hp, mc * P : (mc + 1) * P],
                    qT_sb[dpo : dpo + HS, hp, nb * 512 : (nb + 1) * 512],
                    start=not ident_path,
                    stop=True,
                )
                et = et_pool.tile([P, 512], F16, tag="et", name=f"et_{h}_{mc}")
                if ident_path:
                    nc.scalar.activation(
                        out=et, in_=sp, func=Act.Exp, bias=c_sb[:, mc, h : h + 1]
                    )
                    return et
                e0 = et0_pool.tile([P, 512], F16, tag="et0", name=f"e0_{h}_{mc}")
                nc.scalar.activation(
                    out=e0, in_=sp, func=Act.Exp, bias=c_sb[:, mc, h : h + 1]
                )
                nc.vector.tensor_mul(
                    out=et,
                    in0=e0,
                    in1=eb_t[(h, mc // 2)][:, mc % 2, nb * 512 : (nb + 1) * 512],
                )
                return et

            et_hist = {}
            for mc in range(MC):
                # units 0..1: score->exp(->mult) chains
                for h, nb in units[:2]:
                    et_hist[(h, nb, mc)] = score_chain(h, nb, mc)
                # PE filler: next pair's projection work
                for _ in range(5):
                    if fpos < len(filler):
                        filler[fpos]()
                        fpos += 1
                # eb prefetch: pair 0's second half early in its own loop,
                # the next pair's chunks at the usual cadence
                if hp == 0 and mc in (0, 1):
                    for hn in heads:
                        load_eb(hn, mc + 2)
                if hp < 3 and mc % 2 == 0:
                    for hn in heads:
                        load_eb(hn + 2, mc // 2)
                # PV runs TWO iterations behind its exp/mult chain so the PE
                # never waits on the scalar/vector queues
                if mc >= 2:
                    for h, nb in units:
                        nc.tensor.matmul(
                            f_ps[(h, nb)],
                            v_sb[:, mc - 2, (HS + 1) * h : (HS + 1) * (h + 1)],
                            et_hist.pop((h, nb, mc - 2)),
                            start=(mc - 2 == 0),
                            stop=False,
                        )
                # units 2..3
                for h, nb in units[2:]:
                    et_hist[(h, nb, mc)] = score_chain(h, nb, mc)
            # drain remaining filler, then the two lagged PV batches
            while fpos < len(filler):
                filler[fpos]()
                fpos += 1
            for h, nb in units:
                nc.tensor.matmul(
                    f_ps[(h, nb)],
                    v_sb[:, MC - 2, (HS + 1) * h : (HS + 1) * (h + 1)],
                    et_hist.pop((h, nb, MC - 2)),
                    start=False,
                    stop=False,
                )
            for nbf in range(NB):  # nb0's final PVs (and its norm deps) first
                for h, nb in units:
                    if nb != nbf:
                        continue
                    nc.tensor.matmul(
                        f_ps[(h, nb)],
                        v_sb[:, MC - 1, (HS + 1) * h : (HS + 1) * (h + 1)],
                        et_hist.pop((h, nb, MC - 1)),
                        start=False,
                        stop=True,
                    )
            # normalize; on the last pair, interleave the first-half output
            # projection between the nb=0 and nb=1 norms
            if hp < 3:
                emit_norm(hp, [(h, nb) for nb in range(NB) for h in heads],
                          f_ps, inv32)
            else:
                emit_norm(hp, [(h, 0) for h in heads], f_ps, inv32)
                tags = [("s", 3), ("f", 5)]
                for j, (nt, cb) in enumerate(
                    [(nt, cb) for nt in range(4) for cb in range(2)]
                ):
                    tg, tb = tags[j % 2]
                    emit_group(nt, cb, tg, tb)
                emit_norm(hp, [(h, 1) for h in heads], f_ps, inv32)

        # ---- remaining output projection (needs the final norm) ----
        tags = [("s", 3), ("f", 5)]
        for j, (nt, cb) in enumerate(
            [(nt, cb) for nt in range(4, NT) for cb in range(2)]
        ):
            tg, tb = tags[j % 2]
            emit_group(nt, cb, tg, tb)


def build_program():
    nc = bacc.Bacc("TRN2", target_bir_lowering=False, debug=False, num_devices=8)
    xT = nc.dram_tensor("xT", [H, N], F16, kind="ExternalInput").ap()
    ebT = nc.dram_tensor("ebT", [HL, N, N], F16, kind="ExternalInput").ap()
    wq = nc.dram_tensor("wq", [H, DLOC], F16, kind="ExternalInput").ap()
    wk = nc.dram_tensor("wk", [H, DLOC], F16, kind="ExternalInput").ap()
    wv = nc.dram_tensor("wv", [H, DAUG], F16, kind="ExternalInput").ap()
    wp = nc.dram_tensor("wp", [DLOC, H], F16, kind="ExternalInput").ap()
    bv = nc.dram_tensor("bv", [1, DAUG], F16, kind="ExternalInput").ap()
    ones = nc.dram_tensor("ones", [1, P], F16, kind="ExternalInput").ap()
    onesb2 = nc.dram_tensor("onesb2", [2, P], F32, kind="ExternalInput").ap()
    ident = nc.dram_tensor("ident", [P, P], F16, kind="ExternalInput").ap()
    out = nc.dram_tensor("out", [N, H], F16, kind="ExternalOutput").ap()
    with tile.TileContext(nc) as tc:
        _emit(nc, tc, (xT, ebT, wq, wk, wv, wp, bv, ones, onesb2, ident, out))
    nc.compile()
    return nc


def get_program():
    global _PROG
    if _PROG is None:
        _PROG = build_program()
    return _PROG


def make_in_maps(x, attn_bias, Wq, bq, Wk, bk, Wv, bv, Wp):
    """Host-side sharding: slice/transpose/augment per-core inputs."""
    f = np.float32
    x = np.asarray(x, f)
    attn_bias = np.asarray(attn_bias, f)
    wq_s = np.asarray(Wq, f) * f(SCALE)
    bq_s = np.asarray(bq, f) * f(SCALE)
    Wk = np.asarray(Wk, f)
    Wv, bv = np.asarray(Wv, f), np.asarray(bv, f)
    Wp = np.asarray(Wp, f)

    xTs = [np.ascontiguousarray(x[b].T).astype(np.float16) for b in range(B)]
    onesb2 = np.zeros((2, P), f)
    onesb2[0, :HS] = 1.0
    onesb2[1, HS:] = 1.0
    in_maps = []
    for c in range(8):
        b, g = divmod(c, 2)
        dsl = slice(DLOC * g, DLOC * (g + 1))
        wv_aug = np.zeros((H, DAUG), np.float16)
        bv_aug = np.zeros((1, DAUG), np.float16)
        for hl in range(HL):
            src = slice(DLOC * g + HS * hl, DLOC * g + HS * (hl + 1))
            dst = slice((HS + 1) * hl, (HS + 1) * hl + HS)
            wv_aug[:, dst] = Wv[:, src]
            bv_aug[0, dst] = bv[src]
            bv_aug[0, (HS + 1) * hl + HS] = 1.0
            # c column: exp bias = k0 @ bq for this head (q/k proj biases
            # reduce to this per-key term under the softmax)
            wv_aug[:, CBASE + hl] = (Wk[:, src] @ bq_s[src]).astype(np.float16)
        bT = attn_bias[b, HL * g : HL * (g + 1)].transpose(0, 2, 1)
        ebT = np.exp(bT).astype(np.float16)
        # raw bias rows for the ident path (pairs 1-3 only; pair 0 = heads
        # 0-1 has no predecessor norm to decouple from)
        ebT[2:, : 2 * P, :] = bT[2:, : 2 * P, :]
        in_maps.append(
            {
                "xT": xTs[b],
                "ebT": np.ascontiguousarray(ebT),
                "wq": np.ascontiguousarray(wq_s[:, dsl]).astype(np.float16),
                "wk": np.ascontiguousarray(Wk[:, dsl]).astype(np.float16),
                "wv": wv_aug,
                "wp": np.ascontiguousarray(Wp[dsl, :]).astype(np.float16),
                "bv": bv_aug,
                "ones": np.ones((1, P), np.float16),
                "onesb2": onesb2,
                "ident": np.eye(P, dtype=np.float16),
            }
        )
    return in_maps


def _ensure_ntff_hook():
    """Register the axon NTFF profile hook if the image's antenv lacks it."""
    try:
        from antenv.axon_hooks import get_axon_ntff_profile_hook  # noqa: F401

        return
    except ImportError:
        pass
    import types

    import antenv
    from trn_agent_boot.trn_boot import _ntff_profile_via_ctypes

    mod = types.ModuleType("antenv.axon_hooks")
    box = {"h": None}
    mod.set_axon_ntff_profile_hook = lambda h: box.__setitem__("h", h)
    mod.get_axon_ntff_profile_hook = lambda: box["h"]
    sys.modules["antenv.axon_hooks"] = mod
    antenv.axon_hooks = mod
    hook = _ntff_profile_via_ctypes("/opt/axon/libaxon_pjrt.so")
    if hook is not None:
        mod.set_axon_ntff_profile_hook(hook)


def run_cores(in_maps, trace=False):
    nc = get_program()
    kwargs = {}
    if trace:
        _ensure_ntff_hook()
        kwargs = dict(trace=True, trace_cores=[0])
    return run_bass_kernel_spmd(nc, in_maps, core_ids=list(range(8)), **kwargs)


def kernel(x, attn_bias, Wq, bq, Wk, bk, Wv, bv, Wp, bp):
    in_maps = make_in_maps(x, attn_bias, Wq, bq, Wk, bk, Wv, bv, Wp)
    res = run_cores(in_maps)
    bp = np.asarray(bp, np.float32)
    out = np.empty((B, N, H), np.float32)
    for b in range(B):
        out[b] = (
            res.results[2 * b]["out"].astype(np.float32)
            + res.results[2 * b + 1]["out"].astype(np.float32)
            + bp
        )
    return out
